# revision 14
# baseline (speedup 1.0000x reference)
"""Trainium2 Bass kernel for nn_ComplexFusionModule (dynamic-conv fusion).

Math (reference):
  dw = x1.reshape(B, 4, C1*H*W)                           # (32, 4, 1024)
  k_sum = einsum('bgi,goi->bo', dw, kg_w) + kg_b.sum(0)   # (32, 36864): the 600MB matmul
  kernels = k_sum.reshape(B*C2, C2, 3, 3)
  out1 = per-sample conv3x3(x2, kernels), pad 1
  cw = sigmoid(conv1x1(x1, cw_w) + cw_b)
  o1 = conv1x1(out1 + x2*cw, o1_w) + o1_b
  o2 = conv1x1(x1, o2_w) + o2_b ; o3 = conv1x1(x1, o3_w) + o3_b

Sharding: tensor-parallel over the generator OUT dim (36864 = 64 conv output
channels x 576).  Core c owns conv output channels [8c, 8c+8): it streams a
(4096, 4608) slice of the generator weight (75.5MB/core, the DMA roofline),
computes its k_sum slice for all 32 samples, PE-transposes per-(dydx,co)
blocks (adding the generator bias as a per-partition scalar during the
transpose copy-out), runs the dynamic conv + sigmoid gating for its 8
channels, and emits a partial o1 (o1_w[:, slice] @ fused_slice).  Host sums
the 8 partials.  o2/o3 are batch-sharded (4 samples per core).

Dtypes: the k_sum stream uses float32r (full-rate fp32 streaming, ~1e-4);
all small matmuls (conv taps, gate, o1/o2/o3) are plain fp32 packed 2-4x
into PE column groups via tile_position, which runs at the float32r rate
with exact fp32 results.

Pipelining: the weight slice's columns are ordered (dydx, co, ci) and
streamed in 3 sweeps ([4,4,1] conv-tap groups).  After each sweep the taps
it completes are transposed and their conv partial-products accumulate into
an SBUF out1 accumulator (seeded with the x2*sigmoid gate), interleaved
through the next sweep's weight stream.  The tail is only the last tap's
convs and the o1 matmuls.

Per-core ci rotation: x2's channels (and the matching ci axis of the weight
slice) are rolled so this core's own 8 channels sit at partitions 0..7 —
lets the gating read them from the padded x2 tile without a separate buffer.
"""

import numpy as np

import concourse.bacc as bacc
import concourse.mybir as mybir
import concourse.tile as tile
from concourse.bass_utils import run_bass_kernel_spmd

# dims
B, C1, C2, H, W, KS = 32, 4, 64, 16, 16, 3
IN = C1 * H * W            # 1024
GI = 4 * IN                # 4096 contraction
OUT = C2 * C2 * KS * KS    # 36864
NC = 8                     # cores
CO = C2 // NC              # 8 conv out-channels per core
OPC = CO * C2 * KS * KS    # 4608 per-core OUT slice
HW = H * W                 # 256
HP, WP = H + 2, W + 2      # padded 18x18
BPC = B // NC              # 4 samples per core for o2/o3
KC = GI // 128             # 32 k-chunks

F32 = mybir.dt.float32
F32R = mybir.dt.float32r

SWEEPS = [[0, 1, 2, 3], [4, 5, 6, 7], [8]]  # dydx groups / 512-col psum chunks

_compiled = None
LAST_EXEC_TIME_NS = None
TRACE = False


def _build():
    nc = bacc.Bacc("TRN2", target_bir_lowering=False, debug=False, num_devices=NC)

    # per-core DRAM inputs (k_sum stream fp32r; the rest fp32)
    w2 = nc.dram_tensor("w2", [GI, OPC], F32R, kind="ExternalInput")
    biast = nc.dram_tensor("biast", [C2, KS * KS, CO], F32, kind="ExternalInput")
    dwt = nc.dram_tensor("dwt", [128, KC, B], F32R, kind="ExternalInput")
    x2p = nc.dram_tensor("x2p", [C2, B, HP, WP], F32, kind="ExternalInput")
    x1p = nc.dram_tensor("x1p", [4 * C1, B, H, W], F32, kind="ExternalInput")
    cwt = nc.dram_tensor("cwt", [4 * C1, CO], F32, kind="ExternalInput")
    cwb = nc.dram_tensor("cwb", [CO, 1], F32, kind="ExternalInput")
    o1t = nc.dram_tensor("o1t", [CO, C2], F32, kind="ExternalInput")
    o23t = nc.dram_tensor("o23t", [4 * C1, C1 + 3 * C1], F32, kind="ExternalInput")
    x1o = nc.dram_tensor("x1o", [4 * C1, BPC, HW], F32, kind="ExternalInput")
    ident = nc.dram_tensor("ident", [B, B], F32, kind="ExternalInput")

    # per-core DRAM outputs (o1p: sample pairs packed on the partition dim;
    # o23p: 4 samples x (o2;o3) packed on the partition dim)
    o1p = nc.dram_tensor("o1p", [B // 2, 128, HW], F32, kind="ExternalOutput")
    o23p = nc.dram_tensor("o23p", [128, HW], F32, kind="ExternalOutput")

    with tile.TileContext(nc) as tc:
        with (
            tc.tile_pool(name="consts", bufs=1) as consts,
            tc.tile_pool(name="wpool", bufs=3) as wpool,
            tc.tile_pool(name="work", bufs=1) as work,
            tc.tile_pool(name="ksp", bufs=2) as ksp,
            tc.tile_pool(name="small", bufs=2) as small,
            tc.tile_pool(name="ps1", bufs=4, space="PSUM") as ps1,
            tc.tile_pool(name="ps2", bufs=1, space="PSUM") as ps2,
        ):
            # dwt loads first (gates the very first matmul); the rest of the
            # small inputs load after the first few weight stripes are in flight
            dwt_t = consts.tile([128, KC, B], F32R)
            nc.sync.dma_start(dwt_t[:], dwt[:])
            biast_t = consts.tile([C2, KS * KS, CO], F32)
            ident_t = consts.tile([B, B], F32)
            x2p_t = consts.tile([C2, B, HP, WP], F32)
            x1p_t = consts.tile([4 * C1, B, H, W], F32)
            cwt_t = consts.tile([4 * C1, CO], F32)
            cwb_t = consts.tile([CO, 1], F32)
            o1t_t = consts.tile([CO, C2], F32)
            o23t_t = consts.tile([4 * C1, C1 + 3 * C1], F32)
            x1o_t = consts.tile([4 * C1, BPC, HW], F32)

            def load_misc():
                nc.sync.dma_start(x1p_t[:], x1p[:])
                nc.sync.dma_start(x2p_t[:], x2p[:])
                nc.sync.dma_start(cwt_t[:], cwt[:])
                nc.sync.dma_start(cwb_t[:], cwb[:])
                nc.sync.dma_start(ident_t[:], ident[:])
                nc.sync.dma_start(biast_t[:], biast[:])
                nc.sync.dma_start(o1t_t[:], o1t[:])
                nc.sync.dma_start(o23t_t[:], o23t[:])
                nc.sync.dma_start(x1o_t[:], x1o[:])

            # out1 accumulator: seeded with the gate term x2*sigmoid(...),
            # conv taps accumulate on top across sweeps
            out1sb = work.tile([CO, B, H, W], F32)
            # transposed kernels (+bias): [ci, dydx, co, b]
            ksumT = work.tile([C2, KS * KS, CO, B], F32)

            def make_transpose_item(ksum_s, sweep, dydx, co):
                def emit():
                    off = (dydx - sweep[0]) * 512 + co * C2
                    tp = ps2.tile([C2, B], F32, tag="cw", name="tp")
                    nc.tensor.transpose(tp[:], ksum_s[:, off:off + C2], ident_t[:])
                    nc.vector.tensor_scalar_add(
                        ksumT[:, dydx, co, :], tp[:], biast_t[:, dydx, co:co + 1]
                    )
                return emit

            def make_conv_item(sweep, b0):
                # 4 samples packed into the 4 PE column groups
                def emit():
                    cps = ps2.tile([128, H, W], F32, tag="conv", name="cps")
                    for j, dydx in enumerate(sweep):
                        dy, dx = dydx // KS, dydx % KS
                        for g in range(4):
                            nc.tensor.matmul(
                                cps[32 * g:32 * g + CO],
                                ksumT[:, dydx, :, b0 + g],
                                x2p_t[:, b0 + g, dy:dy + H, dx:dx + W],
                                start=(j == 0),
                                stop=(j == len(sweep) - 1),
                                tile_position=(0, 32 * g),
                            )
                    for g in range(4):
                        nc.vector.tensor_add(
                            out1sb[:, b0 + g], out1sb[:, b0 + g],
                            cps[32 * g:32 * g + CO],
                        )
                return emit

            def emit_cw(b0):
                # gate for 4 samples, packed; seeds out1sb
                gps = ps2.tile([128, H, W], F32, tag="cw", name="gps")
                for g in range(4):
                    nc.tensor.matmul(
                        gps[32 * g:32 * g + CO], cwt_t[:], x1p_t[:, b0 + g],
                        start=True, stop=True, tile_position=(0, 32 * g),
                    )
                for g in range(4):
                    cw_sb = small.tile([CO, H, W], F32, tag="cw_sb", bufs=4)
                    nc.scalar.activation(
                        cw_sb[:], gps[32 * g:32 * g + CO],
                        mybir.ActivationFunctionType.Sigmoid, bias=cwb_t[:],
                    )
                    nc.vector.tensor_mul(
                        out1sb[:, b0 + g], cw_sb[:],
                        x2p_t[0:CO, b0 + g, 1:1 + H, 1:1 + W],
                    )

            pend = []  # deferred transpose+conv work from the previous sweep
            for si, sweep in enumerate(SWEEPS):
                c0 = sweep[0] * 512
                ncol = len(sweep) * 512
                ksum_s = ksp.tile([B, 4 * 512], F32, tag="ksum", name="ksum_s")
                accs = {n: ps1.tile([B, 512], F32, tag="ph1", name=f"acc{n}")
                        for n in sweep}
                npend = len(pend)
                # --- weight stream + k_sum matmuls (+ interleaved work) ---
                for k in range(KC):
                    wt = wpool.tile([128, 4 * 512], F32R, tag="wstripe")
                    nc.sync.dma_start(wt[:, :ncol], w2[k * 128:(k + 1) * 128, c0:c0 + ncol])
                    for n in sweep:
                        nc.tensor.matmul(
                            accs[n][:],
                            dwt_t[:, k, :],
                            wt[:, n * 512 - c0:(n + 1) * 512 - c0],
                            start=(k == 0),
                            stop=(k == KC - 1),
                        )
                    if si == 0 and k == 3:
                        load_misc()
                    if (si == 0 and 8 <= k < 32 and k % 4 == 0) or (si == 1 and k in (0, 4)):
                        emit_cw(k - 8 if si == 0 else 24 + k)
                    if si == 1 and k == 12:
                        # o2/o3 for this core's 4 samples, packed
                        p23 = ps2.tile([128, HW], F32, tag="cw", name="p23")
                        for g in range(BPC):
                            nc.tensor.matmul(
                                p23[32 * g:32 * g + C1 + 3 * C1],
                                o23t_t[:], x1o_t[:, g],
                                start=True, stop=True, tile_position=(0, 32 * g),
                            )
                        o23sb = small.tile([128, HW], F32, tag="o23sb", bufs=1)
                        nc.vector.tensor_copy(o23sb[:], p23[:])
                        nc.sync.dma_start(o23p[:], o23sb[:])
                    for item in pend[npend * k // KC:npend * (k + 1) // KC]:
                        item()
                # copy k_sum out of PSUM
                for j, n in enumerate(sweep):
                    nc.vector.tensor_copy(
                        ksum_s[:, j * 512:(j + 1) * 512], accs[n][:]
                    )
                pend = [make_transpose_item(ksum_s, sweep, dydx, co)
                        for dydx in sweep for co in range(CO)]
                pend += [make_conv_item(sweep, 4 * gi) for gi in range(B // 4)]

            # tail: last sweep's transposes, then conv groups interleaved with
            # the 2-sample-packed o1 partial matmuls
            n_tr = len(SWEEPS[-1]) * CO
            for item in pend[:n_tr]:
                item()

            def emit_o1(pair):
                ops = ps2.tile([128, H, W], F32, tag="conv", name="ops")
                for g in range(2):
                    nc.tensor.matmul(
                        ops[64 * g:64 * g + C2], o1t_t[:], out1sb[:, 2 * pair + g],
                        start=True, stop=True, tile_position=(0, 64 * g),
                    )
                o1sb = small.tile([128, H, W], F32, tag="o1sb")
                nc.vector.tensor_copy(o1sb[:], ops[:])
                nc.sync.dma_start(o1p[pair], o1sb[:].rearrange("p h w -> p (h w)"))

            for gi in range(B // 4):
                pend[n_tr + gi]()
                if gi >= 1:
                    emit_o1(2 * (gi - 1))
                    emit_o1(2 * (gi - 1) + 1)
            emit_o1(B // 2 - 2)
            emit_o1(B // 2 - 1)

    nc.compile()
    return nc


def _prep_inputs(x1, x2, kg_w, kg_b, cw_w, cw_b, o1_w, o2_w, o3_w):
    """Host-side shard/layout prep. Returns per-core input dicts."""
    # dwT in (128, kc, b) chunk-major layout
    dwt = np.ascontiguousarray(
        x1.reshape(B, GI).T.reshape(KC, 128, B).transpose(1, 0, 2)
    )
    x1p = np.ascontiguousarray(x1.reshape(B, 4 * C1, H, W).transpose(1, 0, 2, 3))

    identv = np.eye(B, dtype=np.float32)

    # generator weights viewed as (g, co, ci, ky, kx, i)
    kgw6 = kg_w.reshape(4, C2, C2, KS, KS, IN)
    bias_sum = kg_b.sum(axis=0).reshape(C2, C2, KS, KS)
    o23 = np.ascontiguousarray(np.concatenate([o2_w, o3_w], axis=0).T)  # (16, 16)

    x2pad = np.zeros((B, C2, HP, WP), np.float32)
    x2pad[:, :, 1:H + 1, 1:W + 1] = x2

    per_core = []
    for c in range(NC):
        sl = slice(c * CO, (c + 1) * CO)
        # roll ci so this core's own channels sit first (matched in x2p below)
        perm = np.roll(np.arange(C2), -c * CO)
        # W2 slice: rows (g,i), columns (ky, kx, co_local, ci[perm])
        w2c = np.ascontiguousarray(
            kgw6[:, sl][:, :, perm].transpose(0, 5, 3, 4, 1, 2).reshape(GI, OPC)
        )
        # bias in the transposed domain: [ci, dydx, co]
        biastc = np.ascontiguousarray(
            bias_sum[sl][:, perm].transpose(1, 2, 3, 0).reshape(C2, KS * KS, CO)
        )
        x2pc = np.ascontiguousarray(x2pad[:, perm].transpose(1, 0, 2, 3))
        cwtc = np.ascontiguousarray(cw_w[sl, :].T)
        cwbc = np.ascontiguousarray(cw_b[sl].reshape(CO, 1))
        o1tc = np.ascontiguousarray(o1_w[:, sl].T)
        bsl = slice(c * BPC, (c + 1) * BPC)
        x1oc = np.ascontiguousarray(
            x1[bsl].reshape(BPC, 4 * C1, HW).transpose(1, 0, 2)
        )
        per_core.append({
            "w2": w2c, "biast": biastc, "dwt": dwt, "x2p": x2pc, "x1p": x1p,
            "cwt": cwtc, "cwb": cwbc, "o1t": o1tc, "o23t": o23,
            "x1o": x1oc, "ident": identv,
        })
    return per_core


def kernel(x1, x2, kg_w, kg_b, cw_w, cw_b, o1_w, o1_b, o2_w, o2_b, o3_w, o3_b):
    global _compiled, LAST_EXEC_TIME_NS
    if _compiled is None:
        _compiled = _build()
    nc = _compiled
    in_maps = _prep_inputs(
        np.ascontiguousarray(x1, np.float32), np.ascontiguousarray(x2, np.float32),
        np.ascontiguousarray(kg_w, np.float32), np.asarray(kg_b, np.float32),
        np.asarray(cw_w, np.float32), np.asarray(cw_b, np.float32),
        np.asarray(o1_w, np.float32), np.asarray(o2_w, np.float32),
        np.asarray(o3_w, np.float32),
    )
    res = run_bass_kernel_spmd(nc, in_maps, list(range(NC)), trace=TRACE)
    LAST_EXEC_TIME_NS = res.exec_time_ns

    o1 = np.zeros((B, C2, HW), np.float32)
    for c in range(NC):
        o1 += res.results[c]["o1p"].reshape(B, C2, HW)
    o1 = o1.reshape(B, C2, H, W) + np.asarray(o1_b, np.float32)[None, :, None, None]

    # o23p: (128, HW) = 4 local samples x 32 partitions (16 used)
    o23 = np.concatenate(
        [res.results[c]["o23p"].reshape(BPC, 32, HW)[:, :C1 + 3 * C1] for c in range(NC)],
        axis=0,
    )  # (B, 16, HW)
    o2 = o23[:, :C1].reshape(B, C1, H, W) + np.asarray(o2_b, np.float32)[None, :, None, None]
    o3 = o23[:, C1:].reshape(B, 3 * C1, H, W) + np.asarray(o3_b, np.float32)[None, :, None, None]

    return (np.ascontiguousarray(o1), np.ascontiguousarray(o2), np.ascontiguousarray(o3))


# revision 15
# speedup vs baseline: 1.0869x; 1.0869x over previous
"""Trainium2 Bass kernel for nn_ComplexFusionModule (dynamic-conv fusion).

Math (reference):
  dw = x1.reshape(B, 4, C1*H*W)                           # (32, 4, 1024)
  k_sum = einsum('bgi,goi->bo', dw, kg_w) + kg_b.sum(0)   # (32, 36864): the 600MB matmul
  kernels = k_sum.reshape(B*C2, C2, 3, 3)
  out1 = per-sample conv3x3(x2, kernels), pad 1
  cw = sigmoid(conv1x1(x1, cw_w) + cw_b)
  o1 = conv1x1(out1 + x2*cw, o1_w) + o1_b
  o2 = conv1x1(x1, o2_w) + o2_b ; o3 = conv1x1(x1, o3_w) + o3_b

Sharding: tensor-parallel over the generator OUT dim (36864 = 64 conv output
channels x 576).  Core c owns conv output channels [8c, 8c+8): it streams a
(4096, 4608) slice of the generator weight (75.5MB/core, the DMA roofline),
computes its k_sum slice for all 32 samples, PE-transposes per-(dydx,co)
blocks (adding the generator bias as a per-partition scalar during the
transpose copy-out), runs the dynamic conv + sigmoid gating for its 8
channels, and emits a partial o1 (o1_w[:, slice] @ fused_slice).  Host sums
the 8 partials.  o2/o3 are batch-sharded (4 samples per core).

Dtypes: the k_sum stream uses float32r (full-rate fp32 streaming, ~1e-4);
all small matmuls (conv taps, gate, o1/o2/o3) are plain fp32 packed 2-4x
into PE column groups via tile_position, which runs at the float32r rate
with exact fp32 results.

Pipelining: the weight slice's columns are ordered (dydx, co, ci) and
streamed in 3 sweeps ([4,4,1] conv-tap groups).  After each sweep the taps
it completes are transposed and their conv partial-products accumulate into
an SBUF out1 accumulator (seeded with the x2*sigmoid gate), interleaved
through the next sweep's weight stream.  The tail is only the last tap's
convs and the o1 matmuls.

Per-core ci rotation: x2's channels (and the matching ci axis of the weight
slice) are rolled so this core's own 8 channels sit at partitions 0..7 —
lets the gating read them from the padded x2 tile without a separate buffer.
"""

import numpy as np

import concourse.bacc as bacc
import concourse.mybir as mybir
import concourse.tile as tile
from concourse.bass_utils import run_bass_kernel_spmd

# dims
B, C1, C2, H, W, KS = 32, 4, 64, 16, 16, 3
IN = C1 * H * W            # 1024
GI = 4 * IN                # 4096 contraction
OUT = C2 * C2 * KS * KS    # 36864
NC = 8                     # cores
CO = C2 // NC              # 8 conv out-channels per core
OPC = CO * C2 * KS * KS    # 4608 per-core OUT slice
HW = H * W                 # 256
HP, WP = H + 2, W + 2      # padded 18x18
BPC = B // NC              # 4 samples per core for o2/o3
KC = GI // 128             # 32 k-chunks

F32 = mybir.dt.float32
F32R = mybir.dt.float32r

SWEEPS = [[0, 1, 2, 3], [4, 5, 6, 7], [8]]  # dydx groups / 512-col psum chunks

_compiled = None
LAST_EXEC_TIME_NS = None
TRACE = False


def _build():
    nc = bacc.Bacc("TRN2", target_bir_lowering=False, debug=False, num_devices=NC)

    # per-core DRAM inputs (k_sum stream fp32r; the rest fp32)
    w2 = nc.dram_tensor("w2", [GI, OPC], F32R, kind="ExternalInput")
    biast = nc.dram_tensor("biast", [C2, KS * KS, CO], F32, kind="ExternalInput")
    dwt = nc.dram_tensor("dwt", [128, KC, B], F32R, kind="ExternalInput")
    x2p = nc.dram_tensor("x2p", [C2, B, HP, WP], F32, kind="ExternalInput")
    x1p = nc.dram_tensor("x1p", [4 * C1, B, H, W], F32, kind="ExternalInput")
    cwt = nc.dram_tensor("cwt", [4 * C1, CO], F32, kind="ExternalInput")
    cwb = nc.dram_tensor("cwb", [CO, 1], F32, kind="ExternalInput")
    o1t = nc.dram_tensor("o1t", [CO, C2], F32, kind="ExternalInput")
    o23t = nc.dram_tensor("o23t", [4 * C1, C1 + 3 * C1], F32, kind="ExternalInput")
    x1o = nc.dram_tensor("x1o", [4 * C1, BPC, HW], F32, kind="ExternalInput")
    ident = nc.dram_tensor("ident", [B, B], F32, kind="ExternalInput")

    # per-core DRAM outputs (o1p: sample pairs packed on the partition dim;
    # o23p: 4 samples x (o2;o3) packed on the partition dim)
    o1p = nc.dram_tensor("o1p", [B // 2, 128, HW], F32, kind="ExternalOutput")
    o23p = nc.dram_tensor("o23p", [128, HW], F32, kind="ExternalOutput")

    with tile.TileContext(nc) as tc:
        with (
            tc.tile_pool(name="consts", bufs=1) as consts,
            tc.tile_pool(name="wpool", bufs=3) as wpool,
            tc.tile_pool(name="work", bufs=1) as work,
            tc.tile_pool(name="ksp", bufs=2) as ksp,
            tc.tile_pool(name="small", bufs=2) as small,
            tc.tile_pool(name="ps1", bufs=4, space="PSUM") as ps1,
            tc.tile_pool(name="ps2", bufs=1, space="PSUM") as ps2,
        ):
            # dwt loads first (gates the very first matmul); the rest of the
            # small inputs load after the first few weight stripes are in flight
            dwt_t = consts.tile([128, KC, B], F32R)
            nc.sync.dma_start(dwt_t[:], dwt[:])
            biast_t = consts.tile([C2, KS * KS, CO], F32)
            ident_t = consts.tile([B, B], F32)
            x2p_t = consts.tile([C2, B, HP, WP], F32)
            x1p_t = consts.tile([4 * C1, B, H, W], F32)
            cwt_t = consts.tile([4 * C1, CO], F32)
            cwb_t = consts.tile([CO, 1], F32)
            o1t_t = consts.tile([CO, C2], F32)
            o23t_t = consts.tile([4 * C1, C1 + 3 * C1], F32)
            x1o_t = consts.tile([4 * C1, BPC, HW], F32)

            def load_misc():
                nc.sync.dma_start(x1p_t[:], x1p[:])
                nc.sync.dma_start(x2p_t[:], x2p[:])
                nc.sync.dma_start(cwt_t[:], cwt[:])
                nc.sync.dma_start(cwb_t[:], cwb[:])
                nc.sync.dma_start(ident_t[:], ident[:])
                nc.sync.dma_start(biast_t[:], biast[:])
                nc.sync.dma_start(o1t_t[:], o1t[:])
                nc.sync.dma_start(o23t_t[:], o23t[:])
                nc.sync.dma_start(x1o_t[:], x1o[:])

            # out1 accumulator: seeded with the gate term x2*sigmoid(...),
            # conv taps accumulate on top across sweeps
            out1sb = work.tile([CO, B, H, W], F32)
            # transposed kernels (+bias): [ci, dydx, co, b]
            ksumT = work.tile([C2, KS * KS, CO, B], F32)

            def make_transpose_item(ksum_s, sweep, dydx, co):
                def emit():
                    off = (dydx - sweep[0]) * 512 + co * C2
                    tp = ps2.tile([C2, B], F32, tag="cw", name="tp")
                    nc.tensor.transpose(tp[:], ksum_s[:, off:off + C2], ident_t[:])
                    nc.vector.tensor_scalar_add(
                        ksumT[:, dydx, co, :], tp[:], biast_t[:, dydx, co:co + 1]
                    )
                return emit

            def make_conv_item(sweep, b0):
                # 4 samples packed into the 4 PE column groups
                def emit():
                    cps = ps2.tile([128, H, W], F32, tag="conv", name="cps", bufs=3)
                    for j, dydx in enumerate(sweep):
                        dy, dx = dydx // KS, dydx % KS
                        for g in range(4):
                            nc.tensor.matmul(
                                cps[32 * g:32 * g + CO],
                                ksumT[:, dydx, :, b0 + g],
                                x2p_t[:, b0 + g, dy:dy + H, dx:dx + W],
                                start=(j == 0),
                                stop=(j == len(sweep) - 1),
                                tile_position=(0, 32 * g),
                            )
                    for g in range(4):
                        nc.vector.tensor_add(
                            out1sb[:, b0 + g], out1sb[:, b0 + g],
                            cps[32 * g:32 * g + CO],
                        )
                return emit

            def emit_cw(b0):
                # gate for 4 samples, packed; seeds out1sb
                gps = ps2.tile([128, H, W], F32, tag="cw", name="gps")
                for g in range(4):
                    nc.tensor.matmul(
                        gps[32 * g:32 * g + CO], cwt_t[:], x1p_t[:, b0 + g],
                        start=True, stop=True, tile_position=(0, 32 * g),
                    )
                for g in range(4):
                    cw_sb = small.tile([CO, H, W], F32, tag="cw_sb", bufs=4)
                    nc.scalar.activation(
                        cw_sb[:], gps[32 * g:32 * g + CO],
                        mybir.ActivationFunctionType.Sigmoid, bias=cwb_t[:],
                    )
                    nc.vector.tensor_mul(
                        out1sb[:, b0 + g], cw_sb[:],
                        x2p_t[0:CO, b0 + g, 1:1 + H, 1:1 + W],
                    )

            pend = []  # deferred transpose+conv work from the previous sweep
            for si, sweep in enumerate(SWEEPS):
                c0 = sweep[0] * 512
                ncol = len(sweep) * 512
                ksum_s = ksp.tile([B, 4 * 512], F32, tag="ksum", name="ksum_s")
                accs = {n: ps1.tile([B, 512], F32, tag="ph1", name=f"acc{n}")
                        for n in sweep}
                npend = len(pend)
                # --- weight stream + k_sum matmuls (+ interleaved work) ---
                for k in range(KC):
                    wt = wpool.tile([128, 4 * 512], F32R, tag="wstripe")
                    nc.sync.dma_start(wt[:, :ncol], w2[k * 128:(k + 1) * 128, c0:c0 + ncol])
                    for n in sweep:
                        nc.tensor.matmul(
                            accs[n][:],
                            dwt_t[:, k, :],
                            wt[:, n * 512 - c0:(n + 1) * 512 - c0],
                            start=(k == 0),
                            stop=(k == KC - 1),
                        )
                    if si == 0 and k == 3:
                        load_misc()
                    if (si == 0 and 8 <= k < 32 and k % 4 == 0) or (si == 1 and k in (0, 4)):
                        emit_cw(k - 8 if si == 0 else 24 + k)
                    if si == 1 and k == 12:
                        # o2/o3 for this core's 4 samples, packed
                        p23 = ps2.tile([128, HW], F32, tag="cw", name="p23")
                        for g in range(BPC):
                            nc.tensor.matmul(
                                p23[32 * g:32 * g + C1 + 3 * C1],
                                o23t_t[:], x1o_t[:, g],
                                start=True, stop=True, tile_position=(0, 32 * g),
                            )
                        o23sb = small.tile([128, HW], F32, tag="o23sb", bufs=1)
                        nc.vector.tensor_copy(o23sb[:], p23[:])
                        nc.sync.dma_start(o23p[:], o23sb[:])
                    for item in pend[npend * k // KC:npend * (k + 1) // KC]:
                        item()
                # copy k_sum out of PSUM
                for j, n in enumerate(sweep):
                    nc.vector.tensor_copy(
                        ksum_s[:, j * 512:(j + 1) * 512], accs[n][:]
                    )
                pend = [make_transpose_item(ksum_s, sweep, dydx, co)
                        for dydx in sweep for co in range(CO)]
                pend += [make_conv_item(sweep, 4 * gi) for gi in range(B // 4)]

            # tail: last sweep's transposes, then conv groups interleaved with
            # the 2-sample-packed o1 partial matmuls
            n_tr = len(SWEEPS[-1]) * CO
            for item in pend[:n_tr]:
                item()

            def emit_o1(pair):
                ops = ps2.tile([128, H, W], F32, tag="conv", name="ops", bufs=3)
                for g in range(2):
                    nc.tensor.matmul(
                        ops[64 * g:64 * g + C2], o1t_t[:], out1sb[:, 2 * pair + g],
                        start=True, stop=True, tile_position=(0, 64 * g),
                    )
                o1sb = small.tile([128, H, W], F32, tag="o1sb")
                nc.vector.tensor_copy(o1sb[:], ops[:])
                nc.sync.dma_start(o1p[pair], o1sb[:].rearrange("p h w -> p (h w)"))

            for gi in range(B // 4):
                pend[n_tr + gi]()
                if gi >= 1:
                    emit_o1(2 * (gi - 1))
                    emit_o1(2 * (gi - 1) + 1)
            emit_o1(B // 2 - 2)
            emit_o1(B // 2 - 1)

    nc.compile()
    return nc


def _prep_inputs(x1, x2, kg_w, kg_b, cw_w, cw_b, o1_w, o2_w, o3_w):
    """Host-side shard/layout prep. Returns per-core input dicts."""
    # dwT in (128, kc, b) chunk-major layout
    dwt = np.ascontiguousarray(
        x1.reshape(B, GI).T.reshape(KC, 128, B).transpose(1, 0, 2)
    )
    x1p = np.ascontiguousarray(x1.reshape(B, 4 * C1, H, W).transpose(1, 0, 2, 3))

    identv = np.eye(B, dtype=np.float32)

    # generator weights viewed as (g, co, ci, ky, kx, i)
    kgw6 = kg_w.reshape(4, C2, C2, KS, KS, IN)
    bias_sum = kg_b.sum(axis=0).reshape(C2, C2, KS, KS)
    o23 = np.ascontiguousarray(np.concatenate([o2_w, o3_w], axis=0).T)  # (16, 16)

    x2pad = np.zeros((B, C2, HP, WP), np.float32)
    x2pad[:, :, 1:H + 1, 1:W + 1] = x2

    per_core = []
    for c in range(NC):
        sl = slice(c * CO, (c + 1) * CO)
        # roll ci so this core's own channels sit first (matched in x2p below)
        perm = np.roll(np.arange(C2), -c * CO)
        # W2 slice: rows (g,i), columns (ky, kx, co_local, ci[perm])
        w2c = np.ascontiguousarray(
            kgw6[:, sl][:, :, perm].transpose(0, 5, 3, 4, 1, 2).reshape(GI, OPC)
        )
        # bias in the transposed domain: [ci, dydx, co]
        biastc = np.ascontiguousarray(
            bias_sum[sl][:, perm].transpose(1, 2, 3, 0).reshape(C2, KS * KS, CO)
        )
        x2pc = np.ascontiguousarray(x2pad[:, perm].transpose(1, 0, 2, 3))
        cwtc = np.ascontiguousarray(cw_w[sl, :].T)
        cwbc = np.ascontiguousarray(cw_b[sl].reshape(CO, 1))
        o1tc = np.ascontiguousarray(o1_w[:, sl].T)
        bsl = slice(c * BPC, (c + 1) * BPC)
        x1oc = np.ascontiguousarray(
            x1[bsl].reshape(BPC, 4 * C1, HW).transpose(1, 0, 2)
        )
        per_core.append({
            "w2": w2c, "biast": biastc, "dwt": dwt, "x2p": x2pc, "x1p": x1p,
            "cwt": cwtc, "cwb": cwbc, "o1t": o1tc, "o23t": o23,
            "x1o": x1oc, "ident": identv,
        })
    return per_core


def kernel(x1, x2, kg_w, kg_b, cw_w, cw_b, o1_w, o1_b, o2_w, o2_b, o3_w, o3_b):
    global _compiled, LAST_EXEC_TIME_NS
    if _compiled is None:
        _compiled = _build()
    nc = _compiled
    in_maps = _prep_inputs(
        np.ascontiguousarray(x1, np.float32), np.ascontiguousarray(x2, np.float32),
        np.ascontiguousarray(kg_w, np.float32), np.asarray(kg_b, np.float32),
        np.asarray(cw_w, np.float32), np.asarray(cw_b, np.float32),
        np.asarray(o1_w, np.float32), np.asarray(o2_w, np.float32),
        np.asarray(o3_w, np.float32),
    )
    res = run_bass_kernel_spmd(nc, in_maps, list(range(NC)), trace=TRACE)
    LAST_EXEC_TIME_NS = res.exec_time_ns

    o1 = np.zeros((B, C2, HW), np.float32)
    for c in range(NC):
        o1 += res.results[c]["o1p"].reshape(B, C2, HW)
    o1 = o1.reshape(B, C2, H, W) + np.asarray(o1_b, np.float32)[None, :, None, None]

    # o23p: (128, HW) = 4 local samples x 32 partitions (16 used)
    o23 = np.concatenate(
        [res.results[c]["o23p"].reshape(BPC, 32, HW)[:, :C1 + 3 * C1] for c in range(NC)],
        axis=0,
    )  # (B, 16, HW)
    o2 = o23[:, :C1].reshape(B, C1, H, W) + np.asarray(o2_b, np.float32)[None, :, None, None]
    o3 = o23[:, C1:].reshape(B, 3 * C1, H, W) + np.asarray(o3_b, np.float32)[None, :, None, None]

    return (np.ascontiguousarray(o1), np.ascontiguousarray(o2), np.ascontiguousarray(o3))


# revision 16
# speedup vs baseline: 1.2538x; 1.1536x over previous
"""Trainium2 Bass kernel for nn_ComplexFusionModule (dynamic-conv fusion).

Math (reference):
  dw = x1.reshape(B, 4, C1*H*W)                           # (32, 4, 1024)
  k_sum = einsum('bgi,goi->bo', dw, kg_w) + kg_b.sum(0)   # (32, 36864): the 600MB matmul
  kernels = k_sum.reshape(B*C2, C2, 3, 3)
  out1 = per-sample conv3x3(x2, kernels), pad 1
  cw = sigmoid(conv1x1(x1, cw_w) + cw_b)
  o1 = conv1x1(out1 + x2*cw, o1_w) + o1_b
  o2 = conv1x1(x1, o2_w) + o2_b ; o3 = conv1x1(x1, o3_w) + o3_b

Sharding: tensor-parallel over the generator OUT dim (36864 = 64 conv output
channels x 576).  Core c owns conv output channels [8c, 8c+8): it streams a
(4096, 4608) slice of the generator weight (75.5MB/core, the DMA roofline),
computes its k_sum slice for all 32 samples, PE-transposes per-(dydx,co)
blocks (adding the generator bias as a per-partition scalar during the
transpose copy-out), runs the dynamic conv + sigmoid gating for its 8
channels, and emits a partial o1 (o1_w[:, slice] @ fused_slice).  Host sums
the 8 partials.  o2/o3 are batch-sharded (4 samples per core).

Dtypes: the k_sum stream uses float32r (full-rate fp32 streaming, ~1e-4);
all small matmuls (conv taps, gate, o1/o2/o3) are plain fp32 packed 2-4x
into PE column groups via tile_position, which runs at the float32r rate
with exact fp32 results.

Pipelining: the weight slice's columns are ordered (dydx, co, ci) and
streamed in 3 sweeps ([4,4,1] conv-tap groups).  After each sweep the taps
it completes are transposed and their conv partial-products accumulate into
an SBUF out1 accumulator (seeded with the x2*sigmoid gate), interleaved
through the next sweep's weight stream.  The tail is only the last tap's
convs and the o1 matmuls.

Per-core ci rotation: x2's channels (and the matching ci axis of the weight
slice) are rolled so this core's own 8 channels sit at partitions 0..7 —
lets the gating read them from the padded x2 tile without a separate buffer.
"""

import numpy as np

import concourse.bacc as bacc
import concourse.mybir as mybir
import concourse.tile as tile
from concourse.bass_utils import run_bass_kernel_spmd

# dims
B, C1, C2, H, W, KS = 32, 4, 64, 16, 16, 3
IN = C1 * H * W            # 1024
GI = 4 * IN                # 4096 contraction
OUT = C2 * C2 * KS * KS    # 36864
NC = 8                     # cores
CO = C2 // NC              # 8 conv out-channels per core
OPC = CO * C2 * KS * KS    # 4608 per-core OUT slice
HW = H * W                 # 256
HP, WP = H + 2, W + 2      # padded 18x18
BPC = B // NC              # 4 samples per core for o2/o3
KC = GI // 128             # 32 k-chunks

F32 = mybir.dt.float32
F32R = mybir.dt.float32r

SWEEPS = [[0, 1, 2, 3], [4, 5, 6, 7], [8]]  # dydx groups / 512-col psum chunks

_compiled = None
LAST_EXEC_TIME_NS = None
TRACE = False


def _build():
    nc = bacc.Bacc("TRN2", target_bir_lowering=False, debug=False, num_devices=NC)

    # per-core DRAM inputs (k_sum stream fp32r; the rest fp32)
    w2 = nc.dram_tensor("w2", [GI, OPC], F32R, kind="ExternalInput")
    biast = nc.dram_tensor("biast", [C2, KS * KS, CO], F32, kind="ExternalInput")
    dwt = nc.dram_tensor("dwt", [128, KC, B], F32R, kind="ExternalInput")
    x2p = nc.dram_tensor("x2p", [C2, B, HP, WP], F32, kind="ExternalInput")
    x1p = nc.dram_tensor("x1p", [4 * C1, B, H, W], F32, kind="ExternalInput")
    cwt = nc.dram_tensor("cwt", [4 * C1, CO], F32, kind="ExternalInput")
    cwb = nc.dram_tensor("cwb", [CO, 1], F32, kind="ExternalInput")
    o1t = nc.dram_tensor("o1t", [CO, C2], F32, kind="ExternalInput")
    o23t = nc.dram_tensor("o23t", [4 * C1, C1 + 3 * C1], F32, kind="ExternalInput")
    x1o = nc.dram_tensor("x1o", [4 * C1, BPC, HW], F32, kind="ExternalInput")
    ident = nc.dram_tensor("ident", [B, B], F32, kind="ExternalInput")

    # per-core DRAM outputs (o1p: sample pairs packed on the partition dim;
    # o23p: 4 samples x (o2;o3) packed on the partition dim)
    o1p = nc.dram_tensor("o1p", [B // 2, 128, HW], F32, kind="ExternalOutput")
    o23p = nc.dram_tensor("o23p", [128, HW], F32, kind="ExternalOutput")

    with tile.TileContext(nc) as tc:
        with (
            tc.tile_pool(name="consts", bufs=1) as consts,
            tc.tile_pool(name="wpool", bufs=5) as wpool,
            tc.tile_pool(name="work", bufs=1) as work,
            tc.tile_pool(name="ksp", bufs=2) as ksp,
            tc.tile_pool(name="small", bufs=2) as small,
            tc.tile_pool(name="ps1", bufs=4, space="PSUM") as ps1,
            tc.tile_pool(name="ps2", bufs=1, space="PSUM") as ps2,
        ):
            # dwt loads first (gates the very first matmul); the rest of the
            # small inputs load after the first few weight stripes are in flight
            dwt_t = consts.tile([128, KC, B], F32R)
            nc.sync.dma_start(dwt_t[:], dwt[:])
            biast_t = consts.tile([C2, KS * KS, CO], F32)
            ident_t = consts.tile([B, B], F32)
            x2p_t = consts.tile([C2, B, HP, WP], F32)
            x1p_t = consts.tile([4 * C1, B, H, W], F32)
            cwt_t = consts.tile([4 * C1, CO], F32)
            cwb_t = consts.tile([CO, 1], F32)
            o1t_t = consts.tile([CO, C2], F32)
            o23t_t = consts.tile([4 * C1, C1 + 3 * C1], F32)
            x1o_t = consts.tile([4 * C1, BPC, HW], F32)

            def load_misc():
                nc.sync.dma_start(x1p_t[:], x1p[:])
                nc.sync.dma_start(x2p_t[:], x2p[:])
                nc.sync.dma_start(cwt_t[:], cwt[:])
                nc.sync.dma_start(cwb_t[:], cwb[:])
                nc.sync.dma_start(ident_t[:], ident[:])
                nc.sync.dma_start(biast_t[:], biast[:])
                nc.sync.dma_start(o1t_t[:], o1t[:])
                nc.sync.dma_start(o23t_t[:], o23t[:])
                nc.sync.dma_start(x1o_t[:], x1o[:])

            # out1 accumulator: seeded with the gate term x2*sigmoid(...),
            # conv taps accumulate on top across sweeps
            out1sb = work.tile([CO, B, H, W], F32)
            # transposed kernels (+bias): [ci, dydx, co, b]
            ksumT = work.tile([C2, KS * KS, CO, B], F32)

            def make_transpose_item(ksum_s, sweep, dydx, co):
                def emit():
                    off = (dydx - sweep[0]) * 512 + co * C2
                    tp = ps2.tile([C2, B], F32, tag="conv", name="tp", bufs=3)
                    nc.tensor.transpose(tp[:], ksum_s[:, off:off + C2], ident_t[:])
                    nc.vector.tensor_scalar_add(
                        ksumT[:, dydx, co, :], tp[:], biast_t[:, dydx, co:co + 1]
                    )
                return emit

            def make_conv_item(sweep, b0):
                # 4 samples packed into the 4 PE column groups
                def emit():
                    cps = ps2.tile([128, H, W], F32, tag="conv", name="cps", bufs=3)
                    for j, dydx in enumerate(sweep):
                        dy, dx = dydx // KS, dydx % KS
                        for g in range(4):
                            nc.tensor.matmul(
                                cps[32 * g:32 * g + CO],
                                ksumT[:, dydx, :, b0 + g],
                                x2p_t[:, b0 + g, dy:dy + H, dx:dx + W],
                                start=(j == 0),
                                stop=(j == len(sweep) - 1),
                                tile_position=(0, 32 * g),
                            )
                    for g in range(4):
                        nc.vector.tensor_add(
                            out1sb[:, b0 + g], out1sb[:, b0 + g],
                            cps[32 * g:32 * g + CO],
                        )
                return emit

            def emit_cw(b0):
                # gate for 4 samples, packed; seeds out1sb
                gps = ps2.tile([128, H, W], F32, tag="cw", name="gps")
                for g in range(4):
                    nc.tensor.matmul(
                        gps[32 * g:32 * g + CO], cwt_t[:], x1p_t[:, b0 + g],
                        start=True, stop=True, tile_position=(0, 32 * g),
                    )
                for g in range(4):
                    cw_sb = small.tile([CO, H, W], F32, tag="cw_sb", bufs=4)
                    nc.scalar.activation(
                        cw_sb[:], gps[32 * g:32 * g + CO],
                        mybir.ActivationFunctionType.Sigmoid, bias=cwb_t[:],
                    )
                    nc.vector.tensor_mul(
                        out1sb[:, b0 + g], cw_sb[:],
                        x2p_t[0:CO, b0 + g, 1:1 + H, 1:1 + W],
                    )

            for si, sweep in enumerate(SWEEPS):
                c0 = sweep[0] * 512
                ncol = len(sweep) * 512
                ksum_s = ksp.tile([B, 4 * 512], F32, tag="ksum", name="ksum_s")
                accs = {n: ps1.tile([B, 512], F32, tag="ph1", name=f"acc{n}")
                        for n in sweep}
                # --- weight stream + k_sum matmuls (+ interleaved work) ---
                for k in range(KC):
                    wt = wpool.tile([128, 4 * 512], F32R, tag="wstripe")
                    nc.sync.dma_start(wt[:, :ncol], w2[k * 128:(k + 1) * 128, c0:c0 + ncol])
                    for n in sweep:
                        nc.tensor.matmul(
                            accs[n][:],
                            dwt_t[:, k, :],
                            wt[:, n * 512 - c0:(n + 1) * 512 - c0],
                            start=(k == 0),
                            stop=(k == KC - 1),
                        )
                    if si == 0 and k == 1:
                        load_misc()
                    if si == 0 and 6 <= k <= 27 and k % 3 == 0:
                        emit_cw(4 * (k // 3 - 2))
                    if si == 1 and k == 12:
                        # o2/o3 for this core's 4 samples, packed
                        p23 = ps2.tile([128, HW], F32, tag="cw", name="p23")
                        for g in range(BPC):
                            nc.tensor.matmul(
                                p23[32 * g:32 * g + C1 + 3 * C1],
                                o23t_t[:], x1o_t[:, g],
                                start=True, stop=True, tile_position=(0, 32 * g),
                            )
                        o23sb = small.tile([128, HW], F32, tag="o23sb", bufs=1)
                        nc.vector.tensor_copy(o23sb[:], p23[:])
                        nc.sync.dma_start(o23p[:], o23sb[:])
                # copy k_sum out of PSUM, then transpose + conv this sweep
                for j, n in enumerate(sweep):
                    nc.vector.tensor_copy(
                        ksum_s[:, j * 512:(j + 1) * 512], accs[n][:]
                    )
                for dydx in sweep:
                    for co in range(CO):
                        make_transpose_item(ksum_s, sweep, dydx, co)()
                for gi in range(B // 4):
                    make_conv_item(sweep, 4 * gi)()

            # tail: the 2-sample-packed o1 partial matmuls

            def emit_o1(pair):
                ops = ps2.tile([128, H, W], F32, tag="conv", name="ops", bufs=3)
                for g in range(2):
                    nc.tensor.matmul(
                        ops[64 * g:64 * g + C2], o1t_t[:], out1sb[:, 2 * pair + g],
                        start=True, stop=True, tile_position=(0, 64 * g),
                    )
                o1sb = small.tile([128, H, W], F32, tag="o1sb")
                nc.vector.tensor_copy(o1sb[:], ops[:])
                nc.sync.dma_start(o1p[pair], o1sb[:].rearrange("p h w -> p (h w)"))

            for pair in range(B // 2):
                emit_o1(pair)

    nc.compile()
    return nc


def _prep_inputs(x1, x2, kg_w, kg_b, cw_w, cw_b, o1_w, o2_w, o3_w):
    """Host-side shard/layout prep. Returns per-core input dicts."""
    # dwT in (128, kc, b) chunk-major layout
    dwt = np.ascontiguousarray(
        x1.reshape(B, GI).T.reshape(KC, 128, B).transpose(1, 0, 2)
    )
    x1p = np.ascontiguousarray(x1.reshape(B, 4 * C1, H, W).transpose(1, 0, 2, 3))

    identv = np.eye(B, dtype=np.float32)

    # generator weights viewed as (g, co, ci, ky, kx, i)
    kgw6 = kg_w.reshape(4, C2, C2, KS, KS, IN)
    bias_sum = kg_b.sum(axis=0).reshape(C2, C2, KS, KS)
    o23 = np.ascontiguousarray(np.concatenate([o2_w, o3_w], axis=0).T)  # (16, 16)

    x2pad = np.zeros((B, C2, HP, WP), np.float32)
    x2pad[:, :, 1:H + 1, 1:W + 1] = x2

    per_core = []
    for c in range(NC):
        sl = slice(c * CO, (c + 1) * CO)
        # roll ci so this core's own channels sit first (matched in x2p below)
        perm = np.roll(np.arange(C2), -c * CO)
        # W2 slice: rows (g,i), columns (ky, kx, co_local, ci[perm])
        w2c = np.ascontiguousarray(
            kgw6[:, sl][:, :, perm].transpose(0, 5, 3, 4, 1, 2).reshape(GI, OPC)
        )
        # bias in the transposed domain: [ci, dydx, co]
        biastc = np.ascontiguousarray(
            bias_sum[sl][:, perm].transpose(1, 2, 3, 0).reshape(C2, KS * KS, CO)
        )
        x2pc = np.ascontiguousarray(x2pad[:, perm].transpose(1, 0, 2, 3))
        cwtc = np.ascontiguousarray(cw_w[sl, :].T)
        cwbc = np.ascontiguousarray(cw_b[sl].reshape(CO, 1))
        o1tc = np.ascontiguousarray(o1_w[:, sl].T)
        bsl = slice(c * BPC, (c + 1) * BPC)
        x1oc = np.ascontiguousarray(
            x1[bsl].reshape(BPC, 4 * C1, HW).transpose(1, 0, 2)
        )
        per_core.append({
            "w2": w2c, "biast": biastc, "dwt": dwt, "x2p": x2pc, "x1p": x1p,
            "cwt": cwtc, "cwb": cwbc, "o1t": o1tc, "o23t": o23,
            "x1o": x1oc, "ident": identv,
        })
    return per_core


def kernel(x1, x2, kg_w, kg_b, cw_w, cw_b, o1_w, o1_b, o2_w, o2_b, o3_w, o3_b):
    global _compiled, LAST_EXEC_TIME_NS
    if _compiled is None:
        _compiled = _build()
    nc = _compiled
    in_maps = _prep_inputs(
        np.ascontiguousarray(x1, np.float32), np.ascontiguousarray(x2, np.float32),
        np.ascontiguousarray(kg_w, np.float32), np.asarray(kg_b, np.float32),
        np.asarray(cw_w, np.float32), np.asarray(cw_b, np.float32),
        np.asarray(o1_w, np.float32), np.asarray(o2_w, np.float32),
        np.asarray(o3_w, np.float32),
    )
    res = run_bass_kernel_spmd(nc, in_maps, list(range(NC)), trace=TRACE)
    LAST_EXEC_TIME_NS = res.exec_time_ns

    o1 = np.zeros((B, C2, HW), np.float32)
    for c in range(NC):
        o1 += res.results[c]["o1p"].reshape(B, C2, HW)
    o1 = o1.reshape(B, C2, H, W) + np.asarray(o1_b, np.float32)[None, :, None, None]

    # o23p: (128, HW) = 4 local samples x 32 partitions (16 used)
    o23 = np.concatenate(
        [res.results[c]["o23p"].reshape(BPC, 32, HW)[:, :C1 + 3 * C1] for c in range(NC)],
        axis=0,
    )  # (B, 16, HW)
    o2 = o23[:, :C1].reshape(B, C1, H, W) + np.asarray(o2_b, np.float32)[None, :, None, None]
    o3 = o23[:, C1:].reshape(B, 3 * C1, H, W) + np.asarray(o3_b, np.float32)[None, :, None, None]

    return (np.ascontiguousarray(o1), np.ascontiguousarray(o2), np.ascontiguousarray(o3))
